# revision 1
# baseline (speedup 1.0000x reference)
"""CompressedLinear trn2 kernel.

Computes y = x @ (Q * scales).T + (x @ D.T) @ U.T   for
x [8192, 4096] fp32, Q [4096, 4096] int32 (values 0..126),
scales [4096, 1] fp32, U [4096, 64] fp32, D [64, 4096] fp32.

Strategy: token-parallel over 8 NeuronCores (each core owns 1024 tokens and
computes its full output rows locally; no collectives). Each core:
  - keeps its x.T slice resident in SBUF (16 MiB),
  - streams Q.T in 2 MiB o-panel slabs (one DMA each),
  - computes y.T tiles [128 o, 512 n] on the PE with fp32r matmuls
    (full-rate fp32-precision-ish mode; ~1e-4 scale-relative error),
  - low-rank adapter U @ (D @ x.T) accumulated in a second PSUM bank,
  - epilogue on DVE: out = psum_main * scales[o] + psum_adapter
    (scalar_tensor_tensor with a per-partition scale vector),
  - writes y.T [4096, 1024] to DRAM; host reassembles y.

All host-side work is layout only (transposes/casts into the per-partition-
contiguous layouts the DMAs want); every FLOP of the operator runs on device.
"""

import numpy as np

import concourse.mybir as mybir
import concourse.tile as tile
from concourse import bacc
from concourse import bass_utils as _bass_utils
from concourse.bass_utils import run_bass_kernel_spmd

# Let walrus elide back-to-back LDWEIGHTS with identical weight APs — the
# kernel interleaves both n-blocks per (i, o) weight tile so every stationary
# load is reused by two consecutive matmuls.
LDW_OPT = True

_orig_run_command = _bass_utils.run_command


def _patched_run_command(argv, **kwargs):
    if LDW_OPT:
        argv = [
            a.replace("--enable-ldw-opt=false", "--enable-ldw-opt=true")
            if isinstance(a, str) else a
            for a in argv
        ]
    return _orig_run_command(argv, **kwargs)


_bass_utils.run_command = _patched_run_command

N_TOKENS = 8192
D_IN = 4096
D_OUT = 4096
RANK = 64
N_CORES = 8
N_TOK = N_TOKENS // N_CORES      # 1024 tokens per core
NBLK = 512                       # moving free dim per matmul (PSUM bank)
NB = N_TOK // NBLK               # 2 n-blocks
NI = D_IN // 128                 # 32 contraction tiles
NO = D_OUT // 128                # 32 output-dim tiles
F32R = mybir.dt.float32r
F32 = mybir.dt.float32

_cached_nc = None


def _build():
    nc = bacc.Bacc(None, target_bir_lowering=False)

    # DRAM I/O (per core). float32r is bit-identical to float32.
    xT = nc.dram_tensor("xT", [128, NI * N_TOK], F32R, kind="ExternalInput")
    # Q values are ints in [0, 127) — exact in bf16. Ship bf16 (half the HBM
    # traffic) and upcast to fp32r in-flight via the SWDGE cast DMA.
    q6 = nc.dram_tensor(
        "q6", [NO, 128, NI * 128], mybir.dt.bfloat16, kind="ExternalInput"
    )
    dT = nc.dram_tensor("dT", [128, NI * RANK], F32R, kind="ExternalInput")
    uT = nc.dram_tensor("uT", [NO, RANK, 128], F32R, kind="ExternalInput")
    sc = nc.dram_tensor("sc", [128, NO], F32, kind="ExternalInput")
    yT = nc.dram_tensor("yT", [D_OUT, N_TOK], F32, kind="ExternalOutput")

    with tile.TileContext(nc) as tc:
        with (
            tc.tile_pool(name="xp", bufs=1) as xpool,
            tc.tile_pool(name="qp", bufs=2) as qpool,
            tc.tile_pool(name="dp", bufs=1) as dpool,
            tc.tile_pool(name="up", bufs=2) as upool,
            tc.tile_pool(name="sp", bufs=1) as spool,
            tc.tile_pool(name="tp", bufs=1) as tpool,
            tc.tile_pool(name="op", bufs=2) as opool,
            tc.tile_pool(name="pm", bufs=6, space="PSUM") as psm,
            tc.tile_pool(name="pa", bufs=2, space="PSUM") as psa,
        ):
            sc_sb = spool.tile([128, NO], F32)
            nc.sync.dma_start(sc_sb[:], sc[:])
            dT_sb = dpool.tile([128, NI * RANK], F32R)
            nc.sync.dma_start(dT_sb[:], dT[:])

            # resident x.T, loaded in progressive chunks (small first so the
            # PE can start almost immediately)
            xT_sb = xpool.tile([128, NI * N_TOK], F32R)
            bounds = [0, 1, 2, 4, 8, 14, 20, 26, 32]  # i-tile chunk edges
            for k in range(len(bounds) - 1):
                lo, hi = bounds[k] * N_TOK, bounds[k + 1] * N_TOK
                nc.sync.dma_start(xT_sb[:, lo:hi], xT[:, lo:hi])

            tT_sb = tpool.tile([RANK, N_TOK], F32R)
            state = {}

            def emit_main(ot):
                q_sb = qpool.tile([128, NI * 128], F32R, name="qslab")
                nc.gpsimd.dma_start(q_sb[:], q6[ot])  # SWDGE casts bf16->f32r
                u_sb = upool.tile([RANK, 128], F32R, name="uslab")
                nc.sync.dma_start(u_sb[:], uT[ot])
                # interleave the NB n-blocks so each stationary q tile is
                # reused by NB consecutive matmuls (LDWEIGHTS elided by
                # walrus ldw-opt)
                pms = [
                    psm.tile([128, NBLK], F32, name="pmt") for _ in range(NB)
                ]
                for it in range(NI):
                    for nb in range(NB):
                        nc.tensor.matmul(
                            pms[nb][:],
                            q_sb[:, it * 128:(it + 1) * 128],
                            xT_sb[:, it * N_TOK + nb * NBLK:
                                  it * N_TOK + nb * NBLK + NBLK],
                            start=(it == 0),
                            stop=(it == NI - 1),
                        )
                state[ot] = (pms, u_sb)

            def emit_tail(ot):
                pms, u_sb = state.pop(ot)
                o_sb = opool.tile([128, N_TOK], F32, name="ostage")
                for nb in range(NB):
                    pa = psa.tile([128, NBLK], F32, name="pat")
                    nc.tensor.matmul(
                        pa[:], u_sb[:], tT_sb[:, nb * NBLK:(nb + 1) * NBLK]
                    )
                    # epilogue: ACT does scaled copy of main, DVE adds adapter
                    # (an op may read only ONE non-scalar PSUM input)
                    nc.scalar.mul(
                        o_sb[:, nb * NBLK:(nb + 1) * NBLK],
                        pms[nb][:],
                        sc_sb[:, ot:ot + 1],
                    )
                    nc.vector.tensor_add(
                        o_sb[:, nb * NBLK:(nb + 1) * NBLK],
                        o_sb[:, nb * NBLK:(nb + 1) * NBLK],
                        pa[:],
                    )
                nc.sync.dma_start(yT[ot * 128:(ot + 1) * 128, :], o_sb[:])

            # Head: two o-panels of main MMs keep the PE fed while x.T
            # streams in; the tT groups (which need ALL of x.T) come after
            # them in the PE queue, then their adapters/epilogues.
            NHEAD = 2
            for ot in range(NHEAD):
                emit_main(ot)

            # t.T = D @ x.T  [64, N_TOK], kept resident
            for nb in range(NB):
                pt = psa.tile([RANK, NBLK], F32, name="pat")
                for it in range(NI):
                    nc.tensor.matmul(
                        pt[:],
                        dT_sb[:, it * RANK:(it + 1) * RANK],
                        xT_sb[:, it * N_TOK + nb * NBLK:it * N_TOK + nb * NBLK + NBLK],
                        start=(it == 0),
                        stop=(it == NI - 1),
                    )
                nc.vector.tensor_copy(tT_sb[:, nb * NBLK:(nb + 1) * NBLK], pt[:])

            for ot in range(NHEAD):
                emit_tail(ot)
            for ot in range(NHEAD, NO):
                emit_main(ot)
                emit_tail(ot)

    nc.compile()
    return nc


def kernel(x, scales, U, D, Q, _trace=False, _trace_cores=None):
    global _cached_nc
    if _cached_nc is None:
        _cached_nc = _build()
    nc = _cached_nc

    x = np.asarray(x, dtype=np.float32)
    scales = np.asarray(scales, dtype=np.float32)
    U = np.asarray(U, dtype=np.float32)
    D = np.asarray(D, dtype=np.float32)
    Q = np.asarray(Q)

    # Host layout prep (pure permutation/cast):
    # x7[c, p, it, n] = x[c*N_TOK + n, it*128 + p]
    x7 = np.ascontiguousarray(
        x.reshape(N_CORES, N_TOK, NI, 128).transpose(0, 3, 2, 1)
    ).reshape(N_CORES, 128, NI * N_TOK)
    # q6[ot, p, it, oc] = Q[ot*128 + oc, it*128 + p]; ints < 127 are exact
    # in bf16 (8-bit mantissa)
    import ml_dtypes
    q6 = np.ascontiguousarray(
        Q.reshape(NO, 128, NI, 128).transpose(0, 3, 2, 1).astype(ml_dtypes.bfloat16)
    ).reshape(NO, 128, NI * 128)
    # dT7[p, it, r] = D[r, it*128 + p]
    dT7 = np.ascontiguousarray(
        D.reshape(RANK, NI, 128).transpose(2, 1, 0)
    ).reshape(128, NI * RANK)
    # uT8[ot, r, oc] = U[ot*128 + oc, r]
    uT8 = np.ascontiguousarray(U.reshape(NO, 128, RANK).transpose(0, 2, 1))
    # sc7[p, ot] = scales[ot*128 + p]
    sc7 = np.ascontiguousarray(scales.reshape(NO, 128).T)

    in_maps = [
        {"xT": x7[c], "q6": q6, "dT": dT7, "uT": uT8, "sc": sc7}
        for c in range(N_CORES)
    ]
    kwargs = {}
    if _trace:
        kwargs["trace"] = True
        kwargs["trace_cores"] = _trace_cores or [0]
    r = run_bass_kernel_spmd(nc, in_maps, core_ids=list(range(N_CORES)), **kwargs)
    kernel.last_results = r

    y = np.empty((N_TOKENS, D_OUT), dtype=np.float32)
    for c in range(N_CORES):
        y[c * N_TOK:(c + 1) * N_TOK, :] = r.results[c]["yT"].T
    return y

